# revision 2
# baseline (speedup 1.0000x reference)
"""Trainium2 Bass kernel for nn_HPUAttentionImpl (paged-prompt GQA attention
with alibi + causal bias) running on 8 NeuronCores.

Self-contained: accepts FULL inputs, shards internally, returns FULL outputs.

Design (per core, SPMD — one program, per-core data):
  - 8 "positions", each one (batch, q-head) atom. Atoms are assigned to
    (core, position) so that every position's compile-time tile pattern
    (union over the 8 cores) is as small as possible: alibi decay makes
    far-off-diagonal score tiles negligible, so light heads need few tiles.
  - scores are computed TRANSPOSED [t, s] so that P tiles feed the PV matmul
    directly as the stationary operand (no on-chip transposes anywhere).
  - combined bias factor E = exp(attn_bias + alibi - rowmax) is precomputed
    on host per 128x128 tile (bf16), multiplied into exp(SCALE*QK) on DVE.
    The per-s-column rowmax shift cancels in softmax normalization.
  - P @ [V | 1] yields both the output block and the softmax denominator.
"""

import os

import ml_dtypes
import numpy as np

import concourse.bass as bass
import concourse.mybir as mybir
import concourse.tile as tile
from concourse import bacc
from concourse.bass_utils import run_bass_kernel_spmd

B, S = 2, 1024
H, KV, D = 32, 8, 128
G = H // KV
BLK = 128
NBLOCKS = 32
SCALE = 1.0 / np.sqrt(D)
NT = S // 128          # 8 tiles along s and t
NCORES = 8
NPOS = 8               # atoms per core
SKIP_THR = 45.0        # drop tiles whose bias exponent is below -SKIP_THR

F32 = mybir.dt.float32
F32R = mybir.dt.float32r
F16 = mybir.dt.float16

LAST_RESULTS = None    # BassKernelResults of the most recent run (for test.py)


# --------------------------------------------------------------------------
# Host-side planning
# --------------------------------------------------------------------------

def _zeros_tile():
    return np.zeros((128, 128), dtype=np.float16)


def _build_atoms(attn_bias, alibi_slopes):
    """Per (b, h): dict of active (it, is) -> bf16 E tile in [t, s] layout."""
    pos = np.arange(S, dtype=np.float64)
    rel = pos[None, :] - pos[:, None]          # [s, t] = t - s
    atoms = []
    for b in range(B):
        bias_b = np.asarray(attn_bias[b, 0], dtype=np.float64)
        for h in range(H):
            A = bias_b + float(alibi_slopes[h]) * rel      # [s, t]
            m = A.max(axis=1)                              # [s]
            m = np.where(np.isfinite(m), m, 0.0)
            Ash = A - m[:, None]
            tiles = {}
            for is_ in range(NT):
                sub_rows = Ash[is_ * 128:(is_ + 1) * 128]
                for it in range(NT):
                    sub = sub_rows[:, it * 128:(it + 1) * 128]
                    if sub.max() < -SKIP_THR:
                        continue
                    tiles[(it, is_)] = np.exp(sub).T.astype(np.float16)
            atoms.append({"b": b, "h": h, "kv": h // G, "tiles": tiles})
    return atoms


def _assign(atoms):
    """Assign 64 atoms to (core, position).

    Positions 0-3 of core c hold the 4 q-heads of one "heavy" kv group
    (so K/V is shared); positions 4-7 are filled from the remaining atoms
    band-by-band in decreasing tile count (so per-position union patterns
    stay tight).
    Returns assign[core][pos] -> atom, patterns[pos] -> bool[8,8] (it, is).
    """
    groups = {}
    for a in atoms:
        groups.setdefault((a["b"], a["kv"]), []).append(a)
    gorder = sorted(groups, key=lambda k: -sum(len(a["tiles"]) for a in groups[k]))
    heavy_keys = gorder[:NCORES]
    heavy_set = set(heavy_keys)
    assign = [[None] * NPOS for _ in range(NCORES)]
    for c, gk in enumerate(heavy_keys):
        members = sorted(groups[gk], key=lambda a: -len(a["tiles"]))
        for p in range(4):
            assign[c][p] = members[p]
    rest = [a for a in atoms if (a["b"], a["kv"]) not in heavy_set]
    rest.sort(key=lambda a: -len(a["tiles"]))
    assert len(rest) == NCORES * 4
    for k in range(4):
        band = rest[k * NCORES:(k + 1) * NCORES]
        for c in range(NCORES):
            assign[c][4 + k] = band[c]

    patterns = []
    for p in range(NPOS):
        m = np.zeros((NT, NT), dtype=bool)
        for c in range(NCORES):
            for (it, is_) in assign[c][p]["tiles"]:
                m[it, is_] = True
        patterns.append(m)
    return assign, patterns


def _pattern_geometry(patterns):
    """Per position: active its with bounding is-ranges, and per-is it-lists."""
    geo = []
    for p in range(NPOS):
        m = patterns[p]
        its = []
        for it in range(NT):
            iss = np.nonzero(m[it])[0]
            if len(iss) == 0:
                continue
            its.append((it, int(iss.min()), int(iss.max())))
        is_map = {}
        for (it, lo, hi) in its:
            for is_ in range(lo, hi + 1):
                is_map.setdefault(is_, []).append(it)
        # make sure every is in overall span has at least one it
        geo.append({"its": its, "is_map": sorted(is_map.items())})
    return geo


def _build_e_layout(assign, geo):
    """Decide E-panel layout: try d-indexed prefix panels (dedup), else
    full per-(p,it) panels. Returns (emap {(p,it): (off, ncols)}, tot_cols,
    per-core e_data [128, tot_cols] bf16)."""
    # try causal-style d-indexing: requires is_lo == it for every active it
    dmode = True
    for p in range(NPOS):
        for (it, lo, hi) in geo[p]["its"]:
            if lo != it:
                dmode = False
    if dmode:
        # verify tiles depend only on d = is - it for every core/atom
        for c in range(NCORES):
            for p in range(NPOS):
                tiles = assign[c][p]["tiles"]
                ref = {}
                for (it, lo, hi) in geo[p]["its"]:
                    for is_ in range(lo, hi + 1):
                        d = is_ - it
                        t = tiles.get((it, is_))
                        tb = None if t is None else t.tobytes()
                        if d not in ref:
                            ref[d] = tb
                        elif ref[d] != tb:
                            dmode = False
                if not dmode:
                    break
            if not dmode:
                break

    emap = {}
    off = 0
    if dmode:
        for p in range(NPOS):
            dmax = max(hi - it for (it, _, hi) in geo[p]["its"]) + 1
            base = off
            off += dmax * 128
            for (it, lo, hi) in geo[p]["its"]:
                emap[(p, it)] = (base, (hi - it + 1) * 128)
        tot = off
        e_datas = []
        for c in range(NCORES):
            e = np.zeros((128, tot), dtype=np.float16)
            col = 0
            for p in range(NPOS):
                tiles = assign[c][p]["tiles"]
                dmax = max(hi - it for (it, _, hi) in geo[p]["its"]) + 1
                for d in range(dmax):
                    t = tiles.get((0, d))
                    if t is None:
                        # find any it with this d present (ref equality ensured)
                        for (it, lo, hi) in geo[p]["its"]:
                            if it + d <= hi:
                                t = tiles.get((it, it + d))
                                if t is not None:
                                    break
                    if t is not None:
                        e[:, col:col + 128] = t
                    col += 128
            e_datas.append(e)
        return emap, tot, e_datas

    # general fallback: one panel per (p, it)
    for p in range(NPOS):
        for (it, lo, hi) in geo[p]["its"]:
            emap[(p, it)] = (off, (hi - lo + 1) * 128)
            off += (hi - lo + 1) * 128
    tot = off
    e_datas = []
    for c in range(NCORES):
        e = np.zeros((128, tot), dtype=np.float16)
        for p in range(NPOS):
            tiles = assign[c][p]["tiles"]
            for (it, lo, hi) in geo[p]["its"]:
                base = emap[(p, it)][0]
                for is_ in range(lo, hi + 1):
                    t = tiles.get((it, is_))
                    if t is not None:
                        e[:, base + (is_ - lo) * 128: base + (is_ - lo + 1) * 128] = t
        e_datas.append(e)
    return emap, tot, e_datas


# --------------------------------------------------------------------------
# Device program
# --------------------------------------------------------------------------

def _build_program(geo, emap, e_tot, kv_slot_of_pos, n_kv_slots):
    nc = bacc.Bacc("TRN2", target_bir_lowering=False, debug=False,
                   num_devices=NCORES)
    qT_d = nc.dram_tensor("qT", [NPOS, 128, S], F32, kind="ExternalInput")
    kT_d = nc.dram_tensor("kT", [n_kv_slots, 128, S], F32, kind="ExternalInput")
    vh_d = nc.dram_tensor("vh", [n_kv_slots, NT, 128, 129], F16,
                          kind="ExternalInput")
    e_d = nc.dram_tensor("e", [128, e_tot], F16, kind="ExternalInput")
    cks_d = nc.dram_tensor("cks", [4 * BLK, KV * D], F32, kind="ExternalInput")
    cvs_d = nc.dram_tensor("cvs", [4 * BLK, KV * D], F32, kind="ExternalInput")
    o_d = nc.dram_tensor("o", [NPOS, NT, 128, 128], F32, kind="ExternalOutput")
    cko_d = nc.dram_tensor("cko", [4 * BLK, KV * D], F32, kind="ExternalOutput")
    cvo_d = nc.dram_tensor("cvo", [4 * BLK, KV * D], F32, kind="ExternalOutput")

    with tile.TileContext(nc) as tc:
        with tc.tile_pool(name="eb", bufs=1) as eb_pool, \
             tc.tile_pool(name="qk_sb", bufs=3) as qk_sb_pool, \
             tc.tile_pool(name="vv", bufs=2) as v_pool, \
             tc.tile_pool(name="pt", bufs=2) as p_pool, \
             tc.tile_pool(name="outs", bufs=4) as out_pool, \
             tc.tile_pool(name="rc", bufs=4) as r_pool, \
             tc.tile_pool(name="qk_ps", bufs=2, space="PSUM") as qk_psum_pool, \
             tc.tile_pool(name="o_ps", bufs=3, space="PSUM") as o_psum_pool:

            # paged KV-cache block writes (pure DRAM->DRAM copies)
            nc.sync.dma_start(cko_d.ap()[:], cks_d.ap()[:])
            nc.sync.dma_start(cvo_d.ap()[:], cvs_d.ap()[:])

            e_sb = eb_pool.tile([128, e_tot], F16)
            nc.sync.dma_start(e_sb[:], e_d.ap()[:])

            kv_loaded = {}
            for p in range(NPOS):
                qT = qk_sb_pool.tile([128, S], F32R, tag="qT")
                nc.sync.dma_start(qT[:], qT_d.ap()[p].bitcast(F32R))
                slot = kv_slot_of_pos[p]
                if slot in kv_loaded:
                    kT, vh = kv_loaded[slot]
                else:
                    kT = qk_sb_pool.tile([128, S], F32R, tag=f"kT{slot}")
                    nc.sync.dma_start(kT[:], kT_d.ap()[slot].bitcast(F32R))
                    vh = v_pool.tile([128, NT * 129], F16, tag=f"vh{slot}")
                    # vh dram [NT, 128, 129] -> sbuf [128, NT*129]
                    nc.sync.dma_start(
                        vh[:].rearrange("p (n c) -> p n c", n=NT),
                        vh_d.ap()[slot].rearrange("n p c -> p n c"))
                    kv_loaded[slot] = (kT, vh)

                # ---- QK^T -> exp -> *E  (per active t-tile) ----
                p_tiles = {}
                for (it, lo, hi) in geo[p]["its"]:
                    ncols = (hi - lo + 1) * 128
                    ps = qk_psum_pool.tile([128, 1024], F32)
                    for c0 in range(0, ncols, 512):
                        cw = min(512, ncols - c0)
                        nc.tensor.matmul(
                            ps[:, c0:c0 + cw],
                            kT[:, it * 128:(it + 1) * 128],
                            qT[:, lo * 128 + c0: lo * 128 + c0 + cw],
                            start=True, stop=True)
                    pt = p_pool.tile([128, (NT - it) * 128], F16, tag=f"p{it}")
                    nc.scalar.activation(pt[:, :ncols], ps[:, :ncols],
                                         mybir.ActivationFunctionType.Exp,
                                         scale=float(SCALE))
                    eo, ew = emap[(p, it)]
                    assert ew >= ncols
                    nc.vector.tensor_mul(pt[:, :ncols], pt[:, :ncols],
                                         e_sb[:, eo:eo + ncols])
                    p_tiles[it] = (pt, lo)

                # ---- P @ [V|1] per s-tile, normalize, store ----
                for is_, it_list in geo[p]["is_map"]:
                    po = o_psum_pool.tile([128, 129], F32)
                    for idx, it in enumerate(it_list):
                        pt, lo = p_tiles[it]
                        c0 = (is_ - lo) * 128
                        nc.tensor.matmul(
                            po[:], pt[:, c0:c0 + 128],
                            vh[:, it * 129:(it + 1) * 129],
                            start=(idx == 0), stop=(idx == len(it_list) - 1))
                    rcol = r_pool.tile([128, 1], F32)
                    nc.vector.reciprocal(rcol[:], po[:, 128:129])
                    ob = out_pool.tile([128, 128], F32)
                    nc.any.tensor_scalar_mul(ob[:], po[:, :128], rcol[:])
                    nc.sync.dma_start(o_d.ap()[p, is_], ob[:])
    nc.compile()
    return nc


# --------------------------------------------------------------------------
# Entry point
# --------------------------------------------------------------------------

def kernel(query, key, value, kv_cache, block_indices, attn_bias,
           alibi_slopes):
    global LAST_RESULTS
    query = np.asarray(query, dtype=np.float32)
    key = np.asarray(key, dtype=np.float32)
    value = np.asarray(value, dtype=np.float32)
    kv_cache = np.asarray(kv_cache, dtype=np.float32)
    block_indices = np.asarray(block_indices, dtype=np.int32)
    attn_bias_np = np.asarray(attn_bias, dtype=np.float32)
    alibi_slopes = np.asarray(alibi_slopes, dtype=np.float32)

    atoms = _build_atoms(attn_bias_np, alibi_slopes)
    assign, patterns = _assign(atoms)
    geo = _pattern_geometry(patterns)
    emap, e_tot, e_datas = _build_e_layout(assign, geo)

    # kv slots: positions 0-3 share slot 0 iff all four atoms of every core
    # share a kv head; otherwise every position gets its own slot.
    shared = all(
        len({(assign[c][p]["b"], assign[c][p]["kv"]) for p in range(4)}) == 1
        for c in range(NCORES))
    if shared:
        kv_slot_of_pos = [0, 0, 0, 0, 1, 2, 3, 4]
        n_kv_slots = 5
    else:
        kv_slot_of_pos = list(range(NPOS))
        n_kv_slots = NPOS

    nc = _build_program(geo, emap, e_tot, kv_slot_of_pos, n_kv_slots)

    # ---- per-core input data ----
    q_r = query.reshape(B, S, H, D)
    k_r = key.reshape(B, S, KV, D)
    v_r = value.reshape(B, S, KV, D)

    # cache source blocks: block_indices[i] receives key block i (last wins)
    src_of_block = {}
    for i, blk in enumerate(block_indices.tolist()):
        src_of_block[blk] = i

    in_maps = []
    for c in range(NCORES):
        qT = np.empty((NPOS, 128, S), dtype=np.float32)
        kT = np.empty((n_kv_slots, 128, S), dtype=np.float32)
        vh = np.zeros((n_kv_slots, NT, 128, 129), dtype=np.float16)
        for p in range(NPOS):
            a = assign[c][p]
            qT[p] = q_r[a["b"], :, a["h"], :].T
            slot = kv_slot_of_pos[p]
            kT[slot] = k_r[a["b"], :, a["kv"], :].T
            v = v_r[a["b"], :, a["kv"], :]          # [S, D]
            vt = vh[slot]
            vt[:, :, :D] = v.reshape(NT, 128, D).astype(np.float16)
            vt[:, :, D] = np.asarray(1.0, dtype=np.float16)
        cks = np.empty((4 * BLK, KV * D), dtype=np.float32)
        cvs = np.empty((4 * BLK, KV * D), dtype=np.float32)
        for k_ in range(4):
            blk = 4 * c + k_
            if blk in src_of_block:
                i = src_of_block[blk]
                b_src, j_src = divmod(i, S // BLK)
                cks[k_ * BLK:(k_ + 1) * BLK] = key[b_src,
                                                   j_src * BLK:(j_src + 1) * BLK, :]
                cvs[k_ * BLK:(k_ + 1) * BLK] = value[b_src,
                                                     j_src * BLK:(j_src + 1) * BLK, :]
            else:
                cks[k_ * BLK:(k_ + 1) * BLK] = kv_cache[0, blk].reshape(BLK, KV * D)
                cvs[k_ * BLK:(k_ + 1) * BLK] = kv_cache[1, blk].reshape(BLK, KV * D)
        in_maps.append({
            "qT": qT, "kT": kT, "vh": vh, "e": e_datas[c],
            "cks": cks, "cvs": cvs,
        })

    trace = bool(int(os.environ.get("KERNEL_TRACE", "0")))
    res = run_bass_kernel_spmd(nc, in_maps, core_ids=list(range(NCORES)),
                               trace=trace)
    LAST_RESULTS = res

    # ---- reassemble ----
    out = np.empty((B, S, H, D), dtype=np.float32)
    key_cache = np.empty((NBLOCKS, BLK, KV, D), dtype=np.float32)
    value_cache = np.empty((NBLOCKS, BLK, KV, D), dtype=np.float32)
    for c in range(NCORES):
        r = res.results[c]
        o = r["o"].reshape(NPOS, S, 128)
        for p in range(NPOS):
            a = assign[c][p]
            out[a["b"], :, a["h"], :] = o[p]
        key_cache[4 * c:4 * c + 4] = r["cko"].reshape(4, BLK, KV, D)
        value_cache[4 * c:4 * c + 4] = r["cvo"].reshape(4, BLK, KV, D)
    return out.reshape(B, S, H * D), key_cache, value_cache
